# revision 4
# baseline (speedup 1.0000x reference)
"""Causal self-attention on 8 Trainium2 NeuronCores — v2.

Sharding: tensor-parallel over heads through QKV+attention (2 heads/core).
Output ownership is interleaved per batch: core j owns queries
[j*256, (j+1)*256) of EVERY batch, so a per-batch AllToAll (512KB)
reshards head-parallel y -> token-parallel right after each batch's
attention finishes, hiding all but the last collective under compute.

Pipeline structure (one TileContext, dependency-scheduled):
  QKV(b0) -> [att(b) + QKV(b+1) interleaved per q-block] -> A2A(b)
          -> proj(b-1) emitted after att(b) so its PSUM/PE slot is
             reached only after A2A(b-1) has long completed.

Layouts are transposed throughout ([dim, token]):
  - logits come out as [k, q] -> softmax probs feed the AV matmul
    directly as the moving operand,
  - AV output y^T [hd, q] feeds the projection directly,
  - an all-ones column appended to V computes the softmax denominator
    inside the AV matmul (psum row 64).
Softmax skips max-subtraction (logits are O(+-10); exp in f32 straight
out of PSUM). Compute dtype bf16 (f32 PSUM accumulation).

Queue assignment (collectives block their issuing queue on HW):
  - gpsimd/Pool: collectives + y_recv readbacks + one-time setup only
  - sync:        scatter/broadcast/output-store DMAs
  - scalar/ACT:  exp activations + x-tile loads
  - vector/DVE:  PSUM evacuations, causal-triangle masks, reciprocals
Causal masking is narrowed to the one 128x128 triangle each diagonal
k-tile actually needs (AV reads only [st:QB] per tile, so garbage
columns left of a tile's start are never consumed).
Softmax normalization: one 2-row DVE reciprocal per q-block (a DVE
reciprocal call costs ~3.3us on HW regardless of row count), bounced
through DRAM for the stride-0 64-partition broadcast read, then one
fused multiply into the bf16 scatter tile. A DMA-latency-chained
dummy-matmul keep-alive spans the last AllToAll so the HAM clock gate
doesn't re-throttle the PE before the final projection.
"""

import os

import numpy as np
import ml_dtypes

# Problem dims (nn_CausalSelfAttention: B=4, T=2048, D=1024, H=16)
CFG_FULL = dict(B=4, T=2048, D=1024, H=16)
NCORES = 8
KB = 128  # key tile (partition dim of probs)


def _derived(cfg):
    B, T, D, H = cfg["B"], cfg["T"], cfg["D"], cfg["H"]
    HD = D // H
    assert HD == 64, "design assumes head_dim == 64 (2 heads per 128 partitions)"
    assert H // NCORES == 2, "design assumes 2 heads per core"
    TPB = B * T
    CHUNK = TPB // NCORES   # tokens owned per core (B slices of QT)
    QT = CHUNK // B         # per-batch owned query range
    QB = min(512, T)        # query block (free dim of logits)
    CT = D // 128           # contraction tiles of the model dim
    NQB = T // QB
    assert T % QB == 0 and D % 128 == 0 and TPB % NCORES == 0
    assert QB % KB == 0 and QB == 2 * QT and NQB * 2 == NCORES
    return B, T, D, H, HD, TPB, CHUNK, QT, QB, CT, NQB


def build_nc(cfg=CFG_FULL):
    """Build + compile the (identical-on-every-core) Bass graph."""
    import concourse.bass as bass
    import concourse.tile as tile
    from concourse import bacc
    import concourse.mybir as mybir

    B, T, D, H, HD, TPB, CHUNK, QT, QB, CT, NQB = _derived(cfg)
    f32 = mybir.dt.float32
    bf16 = mybir.dt.bfloat16
    G = QB // KB  # k-tiles per q-block step

    nc = bacc.Bacc("TRN2", target_bir_lowering=False, debug=False,
                   num_devices=NCORES)

    # ---- kernel I/O ----
    xt = nc.dram_tensor("xt", [D, TPB], bf16, kind="ExternalInput")
    wqkvt = nc.dram_tensor("wqkvt", [D, 3 * 128], bf16, kind="ExternalInput")
    wpt = nc.dram_tensor("wpt", [D, D], bf16, kind="ExternalInput")
    out = nc.dram_tensor("out", [D, CHUNK], f32, kind="ExternalOutput")

    # per-batch AllToAll buffers: rows [j*128 + head*64 + d] -> dest core j
    y_send = [nc.dram_tensor(f"y_send_{b}", [NCORES * 128, QT], bf16)
              for b in range(B)]
    y_recv = [nc.dram_tensor(f"y_recv_{b}", [NCORES * 128, QT], bf16)
              for b in range(B)]

    # the one causal triangle every diagonal k-tile needs: m[i,c] = (i <= c)
    tri_np = (np.arange(128)[:, None] <= np.arange(KB)[None, :]) \
        .astype(ml_dtypes.bfloat16)
    tri_dram = nc.inline_tensor(tri_np, name="causal_tri")

    # softmax reciprocals: one 2-row DVE call per q-block (a call costs
    # ~3.3us on HW regardless of row count); the reciprocal rows bounce
    # through DRAM because only DRAM APs allow the stride-0 partition
    # broadcast read
    r_dram = nc.dram_tensor("r_dram", [B, NQB, 2, QB], mybir.dt.float32)

    with tile.TileContext(nc) as tc:
        with (
            tc.tile_pool(name="singles", bufs=1) as singles,
            tc.tile_pool(name="xpool", bufs=3) as xpool,
            tc.tile_pool(name="qk_ps", bufs=2, space="PSUM") as qk_ps,
            tc.tile_pool(name="psy_ps", bufs=2, space="PSUM") as psy_ps,
            tc.tile_pool(name="s_ps", bufs=2, space="PSUM") as s_ps,
            tc.tile_pool(name="ppool", bufs=6) as ppool,
            tc.tile_pool(name="npool", bufs=6) as npool,
            tc.tile_pool(name="ypool", bufs=4) as ypool,
            tc.tile_pool(name="rpool", bufs=2) as rpool,
        ):
            # ---- persistent SBUF ----
            wqkvt_sb = singles.tile([128, CT, 3 * 128], bf16)
            wqr = wqkvt.ap().rearrange("(ct p) o -> p ct o", p=128)
            for ct in range(CT):  # per-ct so the first matmul starts early
                nc.sync.dma_start(out=wqkvt_sb[:, ct:ct + 1, :],
                                  in_=wqr[:, ct:ct + 1, :])
            tri_sb = singles.tile([128, KB], bf16)
            nc.sync.dma_start(out=tri_sb, in_=tri_dram.ap())
            wpt_sb = singles.tile([128, CT, D], bf16)

            q_sb = singles.tile([128, TPB], bf16)   # [2*64 qdim, tok]
            k_sb = singles.tile([128, TPB], bf16)   # [2*64 kdim, tok]
            # V natural + ones column (64) + zero pad to 128 cols so the
            # AV ldweights takes the 4x fast-weight-load path
            v_sb = singles.tile([128, 2, TPB // 128, 128], bf16)
            nc.vector.memset(v_sb[:, :, :, 64:128], 0.0)
            nc.vector.memset(v_sb[:, :, :, 64:65], 1.0)

            # preload the exp table set off the critical path
            warm = singles.tile([1, 2], f32)
            nc.vector.memset(warm, 0.0)
            nc.scalar.activation(out=warm[:, 1:2], in_=warm[:, 0:1],
                                 func=mybir.ActivationFunctionType.Exp)

            xr = xt.ap().rearrange("(ct p) t -> p ct t", p=128)

            def qkv_tt(tt, split_x=False):
                x_sb = xpool.tile([128, CT, 512], bf16, tag="x")
                if split_x:  # per-ct loads so MM(ct0) starts at 1/8 load
                    for ct in range(CT):
                        nc.scalar.dma_start(
                            out=x_sb[:, ct:ct + 1, :],
                            in_=xr[:, ct:ct + 1, tt * 512:(tt + 1) * 512])
                else:
                    nc.scalar.dma_start(
                        out=x_sb, in_=xr[:, :, tt * 512:(tt + 1) * 512])
                # Q^T and K^T: [2 heads * 64 dims, 512 tokens]
                for u, dst in ((0, q_sb), (1, k_sb)):
                    psqk = qk_ps.tile([128, 512], f32, tag="qk")
                    for ct in range(CT):
                        nc.tensor.matmul(
                            psqk,
                            lhsT=wqkvt_sb[:, ct, u * 128:(u + 1) * 128],
                            rhs=x_sb[:, ct, :],
                            start=(ct == 0), stop=(ct == CT - 1))
                    nc.vector.tensor_copy(
                        out=dst[:, tt * 512:(tt + 1) * 512], in_=psqk)
                # V natural: [128 tokens, 2 heads * 64 dims]
                for s4 in range(4):
                    t128 = tt * 4 + s4
                    psv = qk_ps.tile([128, 512], f32, tag="qk")
                    pv = psv[:, 0:128]
                    for ct in range(CT):
                        nc.tensor.matmul(
                            pv,
                            lhsT=x_sb[:, ct, s4 * 128:(s4 + 1) * 128],
                            rhs=wqkvt_sb[:, ct, 256:384],
                            start=(ct == 0), stop=(ct == CT - 1))
                    nc.vector.tensor_copy(
                        out=v_sb[:, :, t128, 0:64],
                        in_=pv.rearrange("p (h d) -> p h d", h=2))

            def attention_qblock(b, qb, dall):
                t0 = b * T
                q0 = t0 + qb * QB
                n_kk = (qb + 1) * G  # causal k-tiles
                n_g = (n_kk + 1) // 2
                psy = [psy_ps.tile([128, QB], f32, tag="psy",
                                   name=f"psy{h}") for h in range(2)]
                p_tiles = []  # (kks, sts, h, p_sb)

                def av(kks, sts, hh, pp):
                    for u, (kk, st) in enumerate(zip(kks, sts)):
                        nc.tensor.matmul(
                            psy[hh][0:128, st:QB],
                            lhsT=v_sb[:, hh, (t0 // 128) + kk, :],
                            rhs=pp[:, u, st:QB],
                            start=(kk == 0), stop=(kk == n_kk - 1))

                for g in range(n_g):
                    kks = [k for k in (g * 2, g * 2 + 1) if k < n_kk]
                    # per-tile valid column start (diagonal narrowing)
                    dls = [k * KB - qb * QB for k in kks]
                    sts = [max(0, d) for d in dls]
                    gst = min(sts)  # group exp column start
                    for h in range(2):
                        hp = h * 64
                        pss = s_ps.tile([128, 2, QB], f32, tag="s")
                        p_sb = ppool.tile([128, 2, QB], bf16, tag="p")
                        for u, kk in enumerate(kks):
                            k0 = t0 + kk * KB
                            nc.tensor.matmul(
                                pss[:, u, gst:QB],
                                lhsT=k_sb[hp:hp + 64, k0:k0 + KB],
                                rhs=q_sb[hp:hp + 64, q0 + gst:q0 + QB],
                                start=True, stop=True)
                        nc.scalar.activation(
                            out=p_sb[:, 0:len(kks), gst:QB],
                            in_=pss[:, 0:len(kks), gst:QB],
                            func=mybir.ActivationFunctionType.Exp,
                            scale=float(HD) ** -0.5)
                        # causal mask: only the KB-wide triangle of each
                        # diagonal tile (AV never reads left of a tile's st)
                        for u in range(len(kks)):
                            dl = dls[u]
                            if dl >= 0:
                                nc.vector.tensor_mul(
                                    p_sb[:, u, dl:dl + KB],
                                    p_sb[:, u, dl:dl + KB], tri_sb)
                        p_tiles.append((kks, sts, h, p_sb))
                        # software-pipeline: AV of group g-1, both heads
                        if g >= 1 and h == 1:
                            for args in p_tiles[-4:-2]:
                                av(*args)
                for args in p_tiles[-2:]:
                    av(*args)

                # evacuate Y psum + stage both heads' denominator rows
                # (SBUF->SBUF) into one 2-row tile, ONE reciprocal call,
                # bounce through DRAM for the stride-0 64-partition
                # broadcast, normalize, scatter. The very last q-block
                # shortens the chain: per-head reciprocal straight from
                # the PSUM denominator row (no staging DMA) — two extra
                # 3.3us DVE calls when DVE is otherwise idle.
                last = (b == B - 1 and qb == NQB - 1)
                y65s = []
                for h in range(2):
                    y65 = ypool.tile([128, QB], f32, tag="y65")
                    nc.vector.tensor_copy(out=y65[0:65, :],
                                          in_=psy[h][0:65, :])
                    if last:  # DVE outs must start at partition 0
                        rf = rpool.tile([1, QB], f32, tag=f"rf{h}")
                        nc.vector.reciprocal(out=rf, in_=psy[h][64:65, :])
                        nc.scalar.dma_start(
                            out=r_dram.ap()[b, qb, h:h + 1, :], in_=rf)
                    else:
                        nc.sync.dma_start(
                            out=dall[h:h + 1, :], in_=y65[64:65, :])
                    y65s.append(y65)
                if not last:
                    rall = rpool.tile([2, QB], f32, tag="rall")
                    nc.vector.reciprocal(out=rall, in_=dall)
                    nc.scalar.dma_start(out=r_dram.ap()[b, qb, :, :],
                                        in_=rall)
                for h in range(2):
                    rb = npool.tile([64, QB], f32, tag="rb")
                    row = r_dram.ap()[b, qb, h:h + 1, :]
                    nc.sync.dma_start(
                        out=rb,
                        in_=bass.AP(tensor=row.tensor, offset=row.offset,
                                    ap=[[0, 64]] + list(row.ap)[1:]))
                    ya = npool.tile([64, QB], bf16, tag="ya")
                    nc.vector.tensor_mul(ya, y65s[h][0:64, :], rb)
                    for cc in range(2):
                        j = 2 * qb + cc
                        r0 = j * 128 + h * 64
                        nc.sync.dma_start(
                            out=y_send[b].ap()[r0:r0 + 64, :],
                            in_=ya[:, cc * QT:(cc + 1) * QT])
                return y65s

            yb_tiles = [singles.tile([128, NCORES, QT], bf16,
                                     name=f"yb_sb{b}") for b in range(B)]

            def proj_batch(b):
                yb_sb = yb_tiles[b]
                for ob in range(D // 128):
                    pso = qk_ps.tile([128, 512], f32, tag="qk")
                    po = pso[:, 0:QT]
                    for i in range(NCORES):
                        nc.tensor.matmul(
                            po,
                            lhsT=wpt_sb[:, i, ob * 128:(ob + 1) * 128],
                            rhs=yb_sb[:, i, :],
                            start=(i == 0), stop=(i == NCORES - 1))
                    o_sb = npool.tile([128, QT], f32, tag="osb")
                    nc.vector.tensor_copy(out=o_sb, in_=po)
                    nc.sync.dma_start(
                        out=out.ap()[ob * 128:(ob + 1) * 128,
                                     b * QT:(b + 1) * QT],
                        in_=o_sb)

            # ---- main pipeline ----
            for tt in range(4):
                qkv_tt(tt, split_x=(tt == 0))
            def readback(b):
                nc.gpsimd.dma_start(
                    out=yb_tiles[b],
                    in_=y_recv[b].ap().rearrange("(i p) t -> p i t", p=128))

            for b in range(B):
                for qb in range(NQB):
                    dall = (None if (b == B - 1 and qb == NQB - 1)
                            else npool.tile([2, QB], f32, tag="dall"))
                    y65s = attention_qblock(b, qb, dall)
                    if b + 1 < B:  # hide next batch's QKV in exp stalls
                        qkv_tt((b + 1) * NQB + qb)
                if b == 0:
                    # wpt fetched during batch-0 attention: off the startup
                    # HBM burst, done before proj(0) needs it
                    nc.gpsimd.dma_start(
                        out=wpt_sb,
                        in_=wpt.ap().rearrange("(ct p) o -> p ct o", p=128))
                nc.gpsimd.collective_compute(
                    "AllToAll", mybir.AluOpType.bypass,
                    replica_groups=[list(range(NCORES))],
                    ins=[y_send[b].ap()], outs=[y_recv[b].ap()])
                readback(b)
                if b >= 2:  # A2A(b-2) completed long ago: zero-stall
                    with tc.high_priority(offset=-100000):
                        proj_batch(b - 2)
            with tc.high_priority(offset=-100000):
                proj_batch(B - 2)
            # HAM keep-alive: the PE idles ~28us during the last AllToAll,
            # which re-throttles the clock gate (K=4/8) and would run
            # proj(B-1) cold. A chain of tiny matmuls, each gated on an
    # SBUF->SBUF DMA (~1.5us latency) fed by the previous one, keeps
            # the array active at < 3us spacing. Pure scratch data.
            ka_a = rpool.tile([1, 64], bf16, tag="ka_a")
            ka_b = rpool.tile([1, 64], bf16, tag="ka_b")
            nc.vector.tensor_scalar_mul(out=ka_a, in0=y65s[1][0:1, 0:64],
                                        scalar1=0.0)  # zeros, att-end dep
            for k in range(16):
                kp = qk_ps.tile([128, 512], f32, tag="qk")
                nc.tensor.matmul(kp[0:64, 0:64], lhsT=ka_a,
                                 rhs=ka_a, start=True, stop=True)
                nc.vector.tensor_copy(out=ka_b, in_=kp[0:1, 0:64])
                nc.scalar.dma_start(out=ka_a, in_=ka_b)
            proj_batch(B - 1)

    nc.compile()
    return nc


def shard_inputs(x, w_qkv, w_proj, cfg=CFG_FULL):
    B, T, D, H, HD, TPB, CHUNK, QT, QB, CT, NQB = _derived(cfg)
    bf16 = ml_dtypes.bfloat16
    xtm = np.ascontiguousarray(
        x.reshape(TPB, D).T).astype(bf16)          # [D, TPB]
    wpt = np.ascontiguousarray(w_proj.T).astype(bf16)  # [D, D]
    in_maps = []
    for i in range(NCORES):
        r = slice(128 * i, 128 * (i + 1))
        wq = w_qkv[0 * D:1 * D][r].T  # [D, 128]
        wk = w_qkv[1 * D:2 * D][r].T
        wv = w_qkv[2 * D:3 * D][r].T
        wqkvt = np.ascontiguousarray(
            np.concatenate([wq, wk, wv], axis=1)).astype(bf16)
        in_maps.append({"xt": xtm, "wqkvt": wqkvt, "wpt": wpt})
    return in_maps


def assemble(outs, cfg=CFG_FULL):
    B, T, D, H, HD, TPB, CHUNK, QT, QB, CT, NQB = _derived(cfg)
    full = np.empty((B, T, D), np.float32)
    for j in range(NCORES):
        o = np.asarray(outs[j], np.float32)  # [D, B*QT], b-major cols
        for b in range(B):
            full[b, j * QT:(j + 1) * QT, :] = o[:, b * QT:(b + 1) * QT].T
    return full


_NC_CACHE = None
last_result = None


def kernel(x, w_qkv, w_proj):
    global _NC_CACHE, last_result
    from concourse.bass_utils import run_bass_kernel_spmd

    if _NC_CACHE is None:
        _NC_CACHE = build_nc()
    in_maps = shard_inputs(np.asarray(x, np.float32),
                           np.asarray(w_qkv, np.float32),
                           np.asarray(w_proj, np.float32))
    trace = os.environ.get("BASS_KERNEL_TRACE", "0") == "1"
    res = run_bass_kernel_spmd(_NC_CACHE, in_maps, list(range(NCORES)),
                               trace=trace)
    last_result = res
    outs = [res.results[i]["out"] for i in range(NCORES)]
    return assemble(outs)


# revision 9
# speedup vs baseline: 1.0700x; 1.0700x over previous
"""Causal self-attention on 8 Trainium2 NeuronCores — v2.

Sharding: tensor-parallel over heads through QKV+attention (2 heads/core).
Output ownership is interleaved per batch: core j owns queries
[j*256, (j+1)*256) of EVERY batch, so a per-batch AllToAll (512KB)
reshards head-parallel y -> token-parallel right after each batch's
attention finishes, hiding all but the last collective under compute.

Pipeline structure (one TileContext, dependency-scheduled):
  QKV(b0) -> [att(b) + QKV(b+1) interleaved per q-block] -> A2A(b)
          -> proj(b-1) emitted after att(b) so its PSUM/PE slot is
             reached only after A2A(b-1) has long completed.

Layouts are transposed throughout ([dim, token]):
  - logits come out as [k, q] -> softmax probs feed the AV matmul
    directly as the moving operand,
  - AV output y^T [hd, q] feeds the projection directly,
  - an all-ones column appended to V computes the softmax denominator
    inside the AV matmul (psum row 64).
Softmax skips max-subtraction (logits are O(+-10); exp in f32 straight
out of PSUM). Compute dtype bf16 (f32 PSUM accumulation).

Queue assignment (collectives block their issuing queue on HW):
  - gpsimd/Pool: collectives + y_recv readbacks + one-time setup only
  - sync:        scatter/broadcast/output-store DMAs
  - scalar/ACT:  exp activations + x-tile loads
  - vector/DVE:  PSUM evacuations, causal-triangle masks, reciprocals
Causal masking is narrowed to the one 128x128 triangle each diagonal
k-tile actually needs (AV reads only [st:QB] per tile, so garbage
columns left of a tile's start are never consumed).
Softmax normalization: one 2-row DVE reciprocal per q-block (a DVE
reciprocal call costs ~3.3us on HW regardless of row count), bounced
through DRAM for the stride-0 64-partition broadcast read, then one
fused multiply into the bf16 scatter tile. A DMA-latency-chained
dummy-matmul keep-alive spans the last AllToAll so the HAM clock gate
doesn't re-throttle the PE before the final projection.
"""

import os

import numpy as np
import ml_dtypes

# Problem dims (nn_CausalSelfAttention: B=4, T=2048, D=1024, H=16)
CFG_FULL = dict(B=4, T=2048, D=1024, H=16)
NCORES = 8
KB = 128  # key tile (partition dim of probs)


def _derived(cfg):
    B, T, D, H = cfg["B"], cfg["T"], cfg["D"], cfg["H"]
    HD = D // H
    assert HD == 64, "design assumes head_dim == 64 (2 heads per 128 partitions)"
    assert H // NCORES == 2, "design assumes 2 heads per core"
    TPB = B * T
    CHUNK = TPB // NCORES   # tokens owned per core (B slices of QT)
    QT = CHUNK // B         # per-batch owned query range
    QB = min(512, T)        # query block (free dim of logits)
    CT = D // 128           # contraction tiles of the model dim
    NQB = T // QB
    assert T % QB == 0 and D % 128 == 0 and TPB % NCORES == 0
    assert QB % KB == 0 and QB == 2 * QT and NQB * 2 == NCORES
    return B, T, D, H, HD, TPB, CHUNK, QT, QB, CT, NQB


def build_nc(cfg=CFG_FULL):
    """Build + compile the (identical-on-every-core) Bass graph."""
    import concourse.bass as bass
    import concourse.tile as tile
    from concourse import bacc
    import concourse.mybir as mybir

    B, T, D, H, HD, TPB, CHUNK, QT, QB, CT, NQB = _derived(cfg)
    f32 = mybir.dt.float32
    bf16 = mybir.dt.bfloat16
    G = QB // KB  # k-tiles per q-block step

    nc = bacc.Bacc("TRN2", target_bir_lowering=False, debug=False,
                   num_devices=NCORES)

    # ---- kernel I/O ----
    xt = nc.dram_tensor("xt", [D, TPB], bf16, kind="ExternalInput")
    wqkvt = nc.dram_tensor("wqkvt", [D, 3 * 128], bf16, kind="ExternalInput")
    wpt = nc.dram_tensor("wpt", [D, D], bf16, kind="ExternalInput")
    out = nc.dram_tensor("out", [D, CHUNK], f32, kind="ExternalOutput")

    # per-batch AllToAll buffers: rows [j*128 + head*64 + d] -> dest core j
    y_send = [nc.dram_tensor(f"y_send_{b}", [NCORES * 128, QT], bf16)
              for b in range(B)]
    y_recv = [nc.dram_tensor(f"y_recv_{b}", [NCORES * 128, QT], bf16)
              for b in range(B)]

    # the one causal triangle every diagonal k-tile needs: m[i,c] = (i <= c)
    tri_np = (np.arange(128)[:, None] <= np.arange(KB)[None, :]) \
        .astype(ml_dtypes.bfloat16)
    tri_dram = nc.inline_tensor(tri_np, name="causal_tri")

    # softmax reciprocals: one 2-row DVE call per q-block (a call costs
    # ~3.3us on HW regardless of row count); the reciprocal rows bounce
    # through DRAM because only DRAM APs allow the stride-0 partition
    # broadcast read
    r_dram = nc.dram_tensor("r_dram", [B, NQB, 2, QB], mybir.dt.float32)

    with tile.TileContext(nc) as tc:
        with (
            tc.tile_pool(name="singles", bufs=1) as singles,
            tc.tile_pool(name="xpool", bufs=4) as xpool,
            tc.tile_pool(name="qk_ps", bufs=2, space="PSUM") as qk_ps,
            tc.tile_pool(name="psy_ps", bufs=2, space="PSUM") as psy_ps,
            tc.tile_pool(name="s_ps", bufs=2, space="PSUM") as s_ps,
            tc.tile_pool(name="ppool", bufs=6) as ppool,
            tc.tile_pool(name="npool", bufs=6) as npool,
            tc.tile_pool(name="ypool", bufs=4) as ypool,
            tc.tile_pool(name="rpool", bufs=2) as rpool,
        ):
            # ---- persistent SBUF ----
            wqkvt_sb = singles.tile([128, CT, 3 * 128], bf16)
            wqr = wqkvt.ap().rearrange("(ct p) o -> p ct o", p=128)
            for ct in range(CT):  # per-ct so the first matmul starts early
                nc.sync.dma_start(out=wqkvt_sb[:, ct:ct + 1, :],
                                  in_=wqr[:, ct:ct + 1, :])
            tri_sb = singles.tile([128, KB], bf16)
            nc.sync.dma_start(out=tri_sb, in_=tri_dram.ap())
            wpt_sb = singles.tile([128, CT, D], bf16)

            q_sb = singles.tile([128, TPB], bf16)   # [2*64 qdim, tok]
            k_sb = singles.tile([128, TPB], bf16)   # [2*64 kdim, tok]
            # V natural + ones column (64) + zero pad to 128 cols so the
            # AV ldweights takes the 4x fast-weight-load path
            v_sb = singles.tile([128, 2, TPB // 128, 128], bf16)
            nc.vector.memset(v_sb[:, :, :, 64:128], 0.0)
            nc.vector.memset(v_sb[:, :, :, 64:65], 1.0)

            # preload the exp table set off the critical path
            warm = singles.tile([1, 2], f32)
            nc.vector.memset(warm, 0.0)
            nc.scalar.activation(out=warm[:, 1:2], in_=warm[:, 0:1],
                                 func=mybir.ActivationFunctionType.Exp)

            xr = xt.ap().rearrange("(ct p) t -> p ct t", p=128)

            def qkv_tt(tt, split_x=False, xq=None):
                # prelude loads ride the idle ACT queue (no exps yet);
                # steady-state ones ride sync so they never block exps
                xq = xq or nc.sync
                x_sb = xpool.tile([128, CT, 512], bf16, tag="x")
                if split_x:  # per-ct loads so MM(ct0) starts at 1/8 load
                    for ct in range(CT):
                        xq.dma_start(
                            out=x_sb[:, ct:ct + 1, :],
                            in_=xr[:, ct:ct + 1, tt * 512:(tt + 1) * 512])
                else:
                    xq.dma_start(
                        out=x_sb, in_=xr[:, :, tt * 512:(tt + 1) * 512])
                # Q^T and K^T: [2 heads * 64 dims, 512 tokens]
                for u, dst in ((0, q_sb), (1, k_sb)):
                    psqk = qk_ps.tile([128, 512], f32, tag="qk")
                    for ct in range(CT):
                        nc.tensor.matmul(
                            psqk,
                            lhsT=wqkvt_sb[:, ct, u * 128:(u + 1) * 128],
                            rhs=x_sb[:, ct, :],
                            start=(ct == 0), stop=(ct == CT - 1))
                    nc.vector.tensor_copy(
                        out=dst[:, tt * 512:(tt + 1) * 512], in_=psqk)
                # V natural: [128 tokens, 2 heads * 64 dims]
                for s4 in range(4):
                    t128 = tt * 4 + s4
                    psv = qk_ps.tile([128, 512], f32, tag="qk")
                    pv = psv[:, 0:128]
                    for ct in range(CT):
                        nc.tensor.matmul(
                            pv,
                            lhsT=x_sb[:, ct, s4 * 128:(s4 + 1) * 128],
                            rhs=wqkvt_sb[:, ct, 256:384],
                            start=(ct == 0), stop=(ct == CT - 1))
                    nc.vector.tensor_copy(
                        out=v_sb[:, :, t128, 0:64],
                        in_=pv.rearrange("p (h d) -> p h d", h=2))

            def attention_qblock(b, qb, dall):
                t0 = b * T
                q0 = t0 + qb * QB
                n_kk = (qb + 1) * G  # causal k-tiles
                n_g = (n_kk + 1) // 2
                psy = [psy_ps.tile([128, QB], f32, tag="psy",
                                   name=f"psy{h}") for h in range(2)]
                p_tiles = []  # (kks, sts, h, p_sb)

                def av(kks, sts, hh, pp):
                    for u, (kk, st) in enumerate(zip(kks, sts)):
                        nc.tensor.matmul(
                            psy[hh][0:128, st:QB],
                            lhsT=v_sb[:, hh, (t0 // 128) + kk, :],
                            rhs=pp[:, u, st:QB],
                            start=(kk == 0), stop=(kk == n_kk - 1))

                for g in range(n_g):
                    kks = [k for k in (g * 2, g * 2 + 1) if k < n_kk]
                    # per-tile valid column start (diagonal narrowing)
                    dls = [k * KB - qb * QB for k in kks]
                    sts = [max(0, d) for d in dls]
                    gst = min(sts)  # group exp column start
                    for h in range(2):
                        hp = h * 64
                        pss = s_ps.tile([128, 2, QB], f32, tag="s")
                        p_sb = ppool.tile([128, 2, QB], bf16, tag="p")
                        for u, kk in enumerate(kks):
                            k0 = t0 + kk * KB
                            nc.tensor.matmul(
                                pss[:, u, gst:QB],
                                lhsT=k_sb[hp:hp + 64, k0:k0 + KB],
                                rhs=q_sb[hp:hp + 64, q0 + gst:q0 + QB],
                                start=True, stop=True)
                        nc.scalar.activation(
                            out=p_sb[:, 0:len(kks), gst:QB],
                            in_=pss[:, 0:len(kks), gst:QB],
                            func=mybir.ActivationFunctionType.Exp,
                            scale=float(HD) ** -0.5)
                        # causal mask: only the KB-wide triangle of each
                        # diagonal tile (AV never reads left of a tile's st)
                        for u in range(len(kks)):
                            dl = dls[u]
                            if dl >= 0:
                                nc.vector.tensor_mul(
                                    p_sb[:, u, dl:dl + KB],
                                    p_sb[:, u, dl:dl + KB], tri_sb)
                        p_tiles.append((kks, sts, h, p_sb))
                        # software-pipeline: AV of group g-1, both heads
                        if g >= 1 and h == 1:
                            for args in p_tiles[-4:-2]:
                                av(*args)
                for args in p_tiles[-2:]:
                    av(*args)

                # evacuate Y psum + stage both heads' denominator rows
                # (SBUF->SBUF) into one 2-row tile, ONE reciprocal call,
                # bounce through DRAM for the stride-0 64-partition
                # broadcast, normalize, scatter. The very last q-block
                # shortens the chain: per-head reciprocal straight from
                # the PSUM denominator row (no staging DMA) — two extra
                # 3.3us DVE calls when DVE is otherwise idle.
                last = (b == B - 1 and qb == NQB - 1)
                y65s = []
                for h in range(2):
                    y65 = ypool.tile([128, QB], f32, tag="y65")
                    nc.vector.tensor_copy(out=y65[0:65, :],
                                          in_=psy[h][0:65, :])
                    if last:  # DVE outs must start at partition 0
                        rf = rpool.tile([1, QB], f32, tag=f"rf{h}")
                        nc.vector.reciprocal(out=rf, in_=psy[h][64:65, :])
                        nc.sync.dma_start(
                            out=r_dram.ap()[b, qb, h:h + 1, :], in_=rf)
                    else:
                        nc.sync.dma_start(
                            out=dall[h:h + 1, :], in_=y65[64:65, :])
                    y65s.append(y65)
                if not last:
                    rall = rpool.tile([2, QB], f32, tag="rall")
                    nc.vector.reciprocal(out=rall, in_=dall)
                    nc.sync.dma_start(out=r_dram.ap()[b, qb, :, :],
                                      in_=rall)
                for h in range(2):
                    rb = npool.tile([64, QB], f32, tag="rb")
                    row = r_dram.ap()[b, qb, h:h + 1, :]
                    nc.sync.dma_start(
                        out=rb,
                        in_=bass.AP(tensor=row.tensor, offset=row.offset,
                                    ap=[[0, 64]] + list(row.ap)[1:]))
                    ya = npool.tile([64, QB], bf16, tag="ya")
                    nc.vector.tensor_mul(ya, y65s[h][0:64, :], rb)
                    for cc in range(2):
                        j = 2 * qb + cc
                        r0 = j * 128 + h * 64
                        nc.sync.dma_start(
                            out=y_send[b].ap()[r0:r0 + 64, :],
                            in_=ya[:, cc * QT:(cc + 1) * QT])
                return y65s

            yb_tiles = [singles.tile([128, NCORES, QT], bf16,
                                     name=f"yb_sb{b}") for b in range(B)]

            def proj_batch(b):
                yb_sb = yb_tiles[b]
                for ob in range(D // 128):
                    pso = qk_ps.tile([128, 512], f32, tag="qk")
                    po = pso[:, 0:QT]
                    for i in range(NCORES):
                        nc.tensor.matmul(
                            po,
                            lhsT=wpt_sb[:, i, ob * 128:(ob + 1) * 128],
                            rhs=yb_sb[:, i, :],
                            start=(i == 0), stop=(i == NCORES - 1))
                    o_sb = npool.tile([128, QT], f32, tag="osb")
                    nc.vector.tensor_copy(out=o_sb, in_=po)
                    nc.sync.dma_start(
                        out=out.ap()[ob * 128:(ob + 1) * 128,
                                     b * QT:(b + 1) * QT],
                        in_=o_sb)

            # ---- main pipeline ----
            for tt in range(4):
                qkv_tt(tt, split_x=(tt == 0), xq=nc.scalar)
            def readback(b):
                nc.gpsimd.dma_start(
                    out=yb_tiles[b],
                    in_=y_recv[b].ap().rearrange("(i p) t -> p i t", p=128))

            for b in range(B):
                for qb in range(NQB):
                    dall = (None if (b == B - 1 and qb == NQB - 1)
                            else npool.tile([2, QB], f32, tag="dall"))
                    y65s = attention_qblock(b, qb, dall)
                    if b + 1 < B:  # hide next batch's QKV in exp stalls
                        qkv_tt((b + 1) * NQB + qb)
                if b == 0:
                    # wpt fetched during batch-0 attention: off the startup
                    # HBM burst, done before proj(0) needs it
                    nc.gpsimd.dma_start(
                        out=wpt_sb,
                        in_=wpt.ap().rearrange("(ct p) o -> p ct o", p=128))
                nc.gpsimd.collective_compute(
                    "AllToAll", mybir.AluOpType.bypass,
                    replica_groups=[list(range(NCORES))],
                    ins=[y_send[b].ap()], outs=[y_recv[b].ap()])
                readback(b)
                if b >= 2:  # A2A(b-2) completed long ago: zero-stall
                    with tc.high_priority(offset=-100000):
                        proj_batch(b - 2)
            with tc.high_priority(offset=-100000):
                proj_batch(B - 2)
            # HAM keep-alive: the PE idles ~28us during the last AllToAll,
            # which re-throttles the clock gate (K=4/8) and would run
            # proj(B-1) cold. A chain of tiny matmuls, each gated on an
    # SBUF->SBUF DMA (~1.5us latency) fed by the previous one, keeps
            # the array active at < 3us spacing. Pure scratch data.
            ka_a = rpool.tile([1, 64], bf16, tag="ka_a")
            ka_b = rpool.tile([1, 64], bf16, tag="ka_b")
            nc.vector.tensor_scalar_mul(out=ka_a, in0=y65s[1][0:1, 0:64],
                                        scalar1=0.0)  # zeros, att-end dep
            for k in range(10):
                kp = qk_ps.tile([128, 512], f32, tag="qk")
                nc.tensor.matmul(kp[0:64, 0:64], lhsT=ka_a,
                                 rhs=ka_a, start=True, stop=True)
                nc.vector.tensor_copy(out=ka_b, in_=kp[0:1, 0:64])
                nc.scalar.dma_start(out=ka_a, in_=ka_b)
            proj_batch(B - 1)

    nc.compile()
    return nc


def shard_inputs(x, w_qkv, w_proj, cfg=CFG_FULL):
    B, T, D, H, HD, TPB, CHUNK, QT, QB, CT, NQB = _derived(cfg)
    bf16 = ml_dtypes.bfloat16
    xtm = np.ascontiguousarray(
        x.reshape(TPB, D).T).astype(bf16)          # [D, TPB]
    wpt = np.ascontiguousarray(w_proj.T).astype(bf16)  # [D, D]
    in_maps = []
    for i in range(NCORES):
        r = slice(128 * i, 128 * (i + 1))
        wq = w_qkv[0 * D:1 * D][r].T  # [D, 128]
        wk = w_qkv[1 * D:2 * D][r].T
        wv = w_qkv[2 * D:3 * D][r].T
        wqkvt = np.ascontiguousarray(
            np.concatenate([wq, wk, wv], axis=1)).astype(bf16)
        in_maps.append({"xt": xtm, "wqkvt": wqkvt, "wpt": wpt})
    return in_maps


def assemble(outs, cfg=CFG_FULL):
    B, T, D, H, HD, TPB, CHUNK, QT, QB, CT, NQB = _derived(cfg)
    full = np.empty((B, T, D), np.float32)
    for j in range(NCORES):
        o = np.asarray(outs[j], np.float32)  # [D, B*QT], b-major cols
        for b in range(B):
            full[b, j * QT:(j + 1) * QT, :] = o[:, b * QT:(b + 1) * QT].T
    return full


_NC_CACHE = None
last_result = None


def kernel(x, w_qkv, w_proj):
    global _NC_CACHE, last_result
    from concourse.bass_utils import run_bass_kernel_spmd

    if _NC_CACHE is None:
        _NC_CACHE = build_nc()
    in_maps = shard_inputs(np.asarray(x, np.float32),
                           np.asarray(w_qkv, np.float32),
                           np.asarray(w_proj, np.float32))
    trace = os.environ.get("BASS_KERNEL_TRACE", "0") == "1"
    res = run_bass_kernel_spmd(_NC_CACHE, in_maps, list(range(NCORES)),
                               trace=trace)
    last_result = res
    outs = [res.results[i]["out"] for i in range(NCORES)]
    return assemble(outs)
